# revision 39
# baseline (speedup 1.0000x reference)
"""Trainium2 Bass kernel for nn_CAM (channel-attention module).

Reference computation per sample (b=16 total):
    xf   = x.reshape(c, h*w)               # [512, 4096]
    attn = softmax(xf @ xf.T, axis=-1)     # [512, 512]
    y    = attn @ xf                       # [512, 4096]
    out  = beta * y + x

Sharding: data-parallel over batch b across 8 NeuronCores (2 samples per
core).  The kernel computes y = softmax(xf xf^T) xf on-chip; the rank-0
epilogue out = x + beta*y runs on the host in fp32 (exact, and it removes
the bf16 x upload + the on-chip elementwise add).

Layout/precision scheme (tolerance 2e-2; matmuls fp8e4 DoubleRow):
  - G = xf xf^T is SYMMETRIC, so P^T[d, c] = exp(G[d,c] - m_c) can be
    built from the natural-layout G tiles with a per-COLUMN max bias --
    no transposes of the 512x512 attention matrix are needed at all.
  - matmul1 (G): xt fp8 (hw-major transpose uploaded from host), 16
    DoubleRow MMs per c-tile; each G tile is row-maxed on the DVE
    (= column max by symmetry) and copied PSUM->SBUF fp32 on the ACT so
    only 2 rotating PSUM banks are needed.  (tensor_tensor_reduce would
    fuse these but crashes the device - verified by bisection.)
  - column-max broadcast: m (shifted by -4096 into bf16 range) is turned
    into M_row[p, c] = m~[c] exactly via 4 diag-mask matmuls
    (lhsT = ones, rhs = identity * m~ per-partition)  -- any bf16
    rounding of m~ cancels between numerator and denominator because the
    row sums are computed FROM the quantized P^T (softmax is invariant
    to a per-row shift applied consistently).
  - P^T tile k = ACT Exp((G_k - 4096) + M_row) -> fp8, values in
    [0, e^~2], safely inside TRN fp8e4 range (max 240).
  - rowsum: 16 N=1 fp8 matmuls P^T(ctile)^T @ ones (~1.3us of PE).  The
    denominator is then the sum of the very fp8 values mm2 multiplies,
    so the P^T quantization cancels in the softmax quotient; an ACT-side
    exp+accum variant was tried and both overloaded the ACT queue (it
    was stalling mm2's drains) and lost the cancellation.
  - matmul2 (y = P @ xf): 2 DR MMs per [128, 512] chunk, 4-chunk groups
    on a stationary weight, two 2-bank PSUM tiles, the first tile's 8 MMs
    emitted before the second's; the drain applies 1/rowsum as a
    per-partition scale on the DVE only (tensor_scalar, bf16 out) -- an
    ACT drain on the PSUM-reuse path repeatedly measured 5-15us slower.
  - PSUM budget: G 2 banks (rotating) + M_row 1 + mm2 4 + rowsum 1 = 8.
  - HBM schedule (per-core DMA BW ~358 GB/s is the scarce resource at
    the start, and each HWDGE queue is packet-rate-bound at ~45
    packets/us): xt is striped across both rings in 4 chunks, x8 rides
    behind it as one 16KB-packet DMA per sample, outputs are one
    8KB-packet DMA per c-tile on alternating rings, and the final
    c-tile ships 1024-col quarters right behind their drains.
  - ~24 identity matmuls run during the initial fill so the PE's HAM
    clock-gate is already at 8/8 when the real matmuls arrive.
  - two-sample software pipeline: sample s+1's matmul1 / sample s's
    matmul2 cover the other sample's softmax tail; emission order is
    arranged so no PE stall exceeds ~0.5us.
"""

import numpy as np
import ml_dtypes

import concourse.bass as bass
import concourse.bacc as bacc
import concourse.mybir as mybir
import concourse.tile as tile
from concourse.bass import ts
from concourse.bass_utils import run_bass_kernel_spmd
from concourse.masks import make_identity

N_CORES = 8
P = 128

F32 = mybir.dt.float32
BF16 = mybir.dt.bfloat16
FP8 = mybir.dt.float8e4

NP_BF16 = ml_dtypes.bfloat16
NP_FP8 = ml_dtypes.float8_e4m3

DR = mybir.MatmulPerfMode.DoubleRow
MULT = mybir.AluOpType.mult
ADD = mybir.AluOpType.add
MAX = mybir.AluOpType.max

# G's diagonal is ||x_c||^2 ~ hw = 4096 for this problem's unit-normal
# input; shifting the column maxes by -4096 keeps them in crisp bf16
# range.  Any residual rounding cancels (see module docstring).
M_SHIFT = 4096.0


def build_program(S=2, C=512, HW=4096, n_cores=N_CORES):
    """Build the SPMD Bass program for one core holding S samples."""
    CT = C // P        # c-tiles (partition tiles of the channel dim)
    NT = HW // P       # n-blocks (contraction tiles for matmul1)
    NCHUNK = 512       # free-dim chunk for matmul2 (one PSUM bank)
    NCH = HW // NCHUNK
    XTC = 4            # xt/x8 arrive in 4 DMA chunks each

    nc = bacc.Bacc(
        "TRN2", target_bir_lowering=False, debug=False, num_devices=n_cores
    )
    # natural x, partition-major, fp8: x8[s, p, i, n] = x[s, 128*i + p, n]
    x8_in = nc.dram_tensor("x8", [S, P, CT, HW], FP8, kind="ExternalInput").ap()
    # transposed x, fp8: xt[s, p, j, c] = x[s, c, 128*j + p]
    xt_in = nc.dram_tensor("xt", [S, P, NT, C], FP8, kind="ExternalInput").ap()
    out_d = nc.dram_tensor("out", [S, P, CT, HW], BF16, kind="ExternalOutput").ap()

    with tile.TileContext(nc) as tc:
        with (
            tc.tile_pool(name="consts", bufs=1) as consts,
            tc.tile_pool(name="xt", bufs=2) as xt_pool,
            tc.tile_pool(name="x8", bufs=2) as x8_pool,
            tc.tile_pool(name="gsb", bufs=2) as gsb_pool,
            tc.tile_pool(name="pt", bufs=2) as pt_pool,
            tc.tile_pool(name="mrow", bufs=2) as mrow_pool,
            tc.tile_pool(name="dsb", bufs=2) as d_pool,
            tc.tile_pool(name="nmd", bufs=8) as nmd_pool,
            tc.tile_pool(name="stats", bufs=2) as stats_pool,
            tc.tile_pool(name="outsb", bufs=3) as out_pool,
            tc.tile_pool(name="psumG", bufs=2, space="PSUM") as psumG_pool,
            tc.tile_pool(name="psumM", bufs=1, space="PSUM") as psumM_pool,
            tc.tile_pool(name="psumY", bufs=1, space="PSUM") as psumY_pool,
            tc.tile_pool(name="psumR", bufs=1, space="PSUM") as psumR_pool,
        ):
            ident = consts.tile([P, P], BF16)
            make_identity(nc, ident[:])
            ones_bf = consts.tile([P, P], BF16)
            nc.vector.memset(ones_bf[:], 1.0)
            ones8 = consts.tile([P, 1], FP8)
            nc.vector.memset(ones8[:], 1.0)

            def warmup_pe(n_mm=24):
                """Identity matmuls during the input fill: trips the HAM
                clock-gate to 8/8 before the real matmul stream starts.
                Output goes to a psumY0-slot tile, long dead before mm2."""
                junk = psumY_pool.tile(
                    [P, 2, NCHUNK], F32, tag="psumY0", name="junk"
                )
                for _ in range(n_mm):
                    nc.tensor.matmul(
                        junk[:, 0, 0:P], lhsT=ident[:], rhs=ident[:],
                        start=True, stop=True,
                    )

            # per-sample state threaded between phases
            st = [dict() for _ in range(S)]

            # Each DMA queue is packet-rate-bound (~45 pkts/us, one packet
            # per partition-row), so per-queue BW ~= pkt_bytes * 45e6/s.
            # Stripe xt's chunks across both HWDGE rings and keep the
            # packet (= per-partition contiguous run) as big as the
            # consumption order allows; x8 is one 16KB-packet DMA per
            # sample, queued behind xt on its ring so it never steals BW.
            XT_EDGES = [0, 4, 12, 22, NT]

            def load_xt(s, engs=(nc.sync, nc.scalar)):
                xt_t = xt_pool.tile([P, NT, C], FP8, tag="xt")
                for c in range(len(XT_EDGES) - 1):
                    lo, hi = XT_EDGES[c], XT_EDGES[c + 1]
                    engs[c % 2].dma_start(
                        xt_t[:, lo:hi, :], xt_in[s, :, lo:hi, :]
                    )
                st[s]["xt"] = xt_t

            def load_x8(s, eng):
                x8_t = x8_pool.tile([P, CT, HW], FP8, tag="x8", name="x8_t")
                eng.dma_start(x8_t[:, :, :], x8_in[s])
                st[s]["x8"] = x8_t

            def alloc_mm1(s):
                st[s]["gsb"] = gsb_pool.tile([P, CT, C], F32, tag="gsb", name="gsb")
                st[s]["negm"] = stats_pool.tile(
                    [P, CT], F32, tag="negm", name="negm"
                )
                st[s]["nm"] = stats_pool.tile([P, CT], F32, tag="nm", name="nm")

            def mm1_tile(s, i):
                """G c-tile i: 16 DR MMs -> PSUM; rowmax (DVE) + fp32 copy
                (ACT) drain the bank."""
                xt_t, gsb, negm = st[s]["xt"], st[s]["gsb"], st[s]["negm"]
                pa = psumG_pool.tile([P, C], F32, tag="psumG", name="pa")
                for t in range(NT // 2):
                    nc.tensor.matmul(
                        pa[:],
                        lhsT=xt_t[:, 2 * t : 2 * t + 2, ts(i, P)],
                        rhs=xt_t[:, 2 * t : 2 * t + 2, :],
                        start=(t == 0),
                        stop=(t == NT // 2 - 1),
                        perf_mode=DR,
                    )
                # by symmetry rowmax == colmax
                nc.vector.reduce_max(
                    negm[:, i : i + 1], pa[:], axis=mybir.AxisListType.X,
                    negate=True,
                )
                nc.scalar.copy(gsb[:, i, :], pa[:])
                if i == CT - 1:
                    # m~ = bf16-round(M_SHIFT - max) in an fp32 view; any
                    # rounding cancels between P^T and the rowsums because
                    # both derive from the same broadcast values.
                    nm_bf = stats_pool.tile([P, CT], BF16, name="nm_bf")
                    nc.vector.tensor_scalar_add(
                        nm_bf[:], st[s]["negm"][:], M_SHIFT
                    )
                    nc.vector.tensor_copy(st[s]["nm"][:], nm_bf[:])

            def nmd_phase(s):
                """nmd_j = I * nm[:, j] (DVE) - split from the matmuls so
                these can be emitted ahead of mm2's DVE drains."""
                nm = st[s]["nm"]
                nmds = []
                for j in range(CT):
                    nmd = nmd_pool.tile([P, P], BF16, tag="nmd", name="nmd")
                    nc.vector.tensor_scalar_mul(
                        nmd[:], ident[:], nm[:, j : j + 1]
                    )
                    nmds.append(nmd)
                st[s]["nmds"] = nmds

            def mrow_phase(s):
                """M_row[p, c] = nm[c] for all p, via 4 diag-mask matmuls."""
                pm = psumM_pool.tile([P, C], F32, tag="psumM", name="pm")
                mrow = mrow_pool.tile([P, C], F32, tag="mrow")
                for j in range(CT):
                    # out[p, q] = sum_k 1 * (I[k,q] * nm[k, j]) = nm[q, j]
                    nc.tensor.matmul(
                        pm[:, ts(j, P)], lhsT=ones_bf[:], rhs=st[s]["nmds"][j][:],
                        start=True, stop=True,
                    )
                nc.vector.tensor_copy(mrow[:], pm[:])
                st[s]["mrow"] = mrow

            def softmax_tiles(s, ks=None):
                """PT tile k = exp(G_k - m_col), fp8."""
                gsb, mrow = st[s]["gsb"], st[s]["mrow"]
                if ks is None:
                    ks = range(CT)
                if 0 in ks:
                    st[s]["PT"] = pt_pool.tile(
                        [P, CT, C], FP8, tag="PT", name="PT"
                    )
                PT = st[s]["PT"]
                for k in ks:
                    d_t = d_pool.tile([P, C], BF16, tag="dsb")
                    nc.vector.scalar_tensor_tensor(
                        out=d_t[:],
                        in0=gsb[:, k, :],
                        scalar=-M_SHIFT,
                        in1=mrow[:],
                        op0=ADD,
                        op1=ADD,
                    )
                    nc.scalar.activation(
                        PT[:, k, :], d_t[:], mybir.ActivationFunctionType.Exp
                    )

            def rowsum_phase(s):
                """rowsum[c] = sum_d PT[d, c] via N=1 fp8 matmuls; the
                denominator then exactly matches mm2's quantized numerator."""
                PT = st[s]["PT"]
                rs = psumR_pool.tile([P, CT], F32, tag="psumR", name="rs")
                rsinv = stats_pool.tile([P, CT], F32, tag="rsinv", name="rsinv")
                for i in range(CT):
                    for k in range(CT):
                        nc.tensor.matmul(
                            rs[:, i : i + 1],
                            lhsT=PT[:, k, ts(i, P)],
                            rhs=ones8[:],
                            start=(k == 0),
                            stop=(k == CT - 1),
                        )
                nc.vector.reciprocal(rsinv[:], rs[:])
                st[s]["rsinv"] = rsinv

            def mm2_phase(s, tiles):
                x8_t, PT = st[s]["x8"], st[s]["PT"]
                last = s == S - 1 and CT - 1 in tiles
                for i in tiles:
                    rsinv = st[s]["rsinv"]
                    fine = last and i == CT - 1   # kernel-tail tile
                    ot = out_pool.tile([P, HW], BF16, tag="outsb")
                    for g in range(NCH // 4):
                        pys = [
                            psumY_pool.tile(
                                [P, 2, NCHUNK], F32, tag=f"psumY{q}", name=f"py{q}"
                            )
                            for q in range(2)
                        ]
                        # q-blocked emission: pys[0]'s 8 MMs all run before
                        # pys[1]'s, so each tile's DVE drain starts half a
                        # group early and finishes under the other half's
                        # MMs -- the bank reuse never waits.  Both drains go
                        # to the DVE: the ACT queue's in-order latency was
                        # measured adding ~1.7us/group when it held a drain.
                        for q in range(2):
                            for t in range(CT // 2):
                                for j in range(2):
                                    n = g * 4 + q * 2 + j
                                    nc.tensor.matmul(
                                        pys[q][:, j, :],
                                        lhsT=PT[:, 2 * t : 2 * t + 2, ts(i, P)],
                                        rhs=x8_t[:, 2 * t : 2 * t + 2, ts(n, NCHUNK)],
                                        start=(t == 0),
                                        stop=(t == CT // 2 - 1),
                                        perf_mode=DR,
                                    )
                        # both drains on the DVE: an ACT drain anywhere on
                        # the bank-reuse path measured ~5-15us slower end to
                        # end across three attempts (the ACT's in-order
                        # queue adds ~0.9-1.7us of latency per group).
                        for q in range(2):
                            nc.vector.tensor_scalar_mul(
                                ot[:, ts(2 * g + q, 2 * NCHUNK)],
                                pys[q][:],
                                rsinv[:, i : i + 1],
                            )
                        if fine:
                            # kernel tail: ship each 1024-col quarter as
                            # soon as its drain lands, alternating rings
                            for h in (2 * g, 2 * g + 1):
                                eng = nc.sync if h % 2 == 0 else nc.scalar
                                eng.dma_start(
                                    out_d[s, :, i, ts(h, NCHUNK * 2)],
                                    ot[:, ts(h, NCHUNK * 2)],
                                )
                    if not fine:
                        # one 8KB-packet DMA per c-tile, rings alternating
                        eng = nc.sync if (s * CT + i) % 2 == 0 else nc.scalar
                        eng.dma_start(out_d[s, :, i, :], ot[:])

            # -- software-pipelined emission over the S=2 samples --
            warmup_pe()
            load_xt(0)                     # striped over both rings
            load_xt(1, (nc.scalar, nc.sync))
            load_x8(0, nc.sync)            # FIFO behind xt on each ring
            load_x8(1, nc.scalar)

            alloc_mm1(0)
            for i in range(CT):
                mm1_tile(0, i)
            alloc_mm1(1)
            mm1_tile(1, 0)
            nmd_phase(0)
            mrow_phase(0)          # PE: 4 tiny MMs, hidden under mm1(1)
            softmax_tiles(0)       # DVE+ACT, hidden under mm1(1)
            mm1_tile(1, 1)
            mm1_tile(1, 2)
            mm1_tile(1, 3)
            rowsum_phase(0)        # PE: 16 tiny MMs
            # nmd(1) DVE ops enter the DVE queue before mm2(0)'s drains so
            # the bcast MMs emitted mid-mm2 never stall the PE
            nmd_phase(1)
            mm2_phase(0, [0, 1])
            mrow_phase(1)
            # softmax(1)'s DVE subtracts are interleaved between mm2(0)
            # tiles so they never head-of-line-block the mm2 drains
            softmax_tiles(1, ks=[0, 1])
            mm2_phase(0, [2])
            softmax_tiles(1, ks=[2, 3])
            mm2_phase(0, [3])
            rowsum_phase(1)
            mm2_phase(1, [0, 1, 2, 3])

    nc.compile()
    return nc


_PROGRAM_CACHE = {}


def _get_program(S, C, HW, n_cores):
    key = (S, C, HW, n_cores)
    if key not in _PROGRAM_CACHE:
        _PROGRAM_CACHE[key] = build_program(S, C, HW, n_cores)
    return _PROGRAM_CACHE[key]


def make_in_maps(x: np.ndarray):
    """Host-side prep: shard over batch, swizzle + downcast to fp8."""
    b, c, h, w = x.shape
    hw = h * w
    S = b // N_CORES
    CT = c // P
    NT = hw // P

    xf = np.asarray(x, dtype=np.float32).reshape(b, c, hw)
    # natural, partition-major: [b, P, CT, HW]
    x8 = np.ascontiguousarray(
        xf.reshape(b, CT, P, hw).transpose(0, 2, 1, 3)
    ).astype(NP_FP8)
    # transposed: xt[s, p, j, c] = x[s, c, 128j+p] -> [b, P, NT, C]
    xt = np.ascontiguousarray(
        xf.reshape(b, c, NT, P).transpose(0, 3, 2, 1)
    ).astype(NP_FP8)
    return [
        {
            "x8": x8[core * S : (core + 1) * S],
            "xt": xt[core * S : (core + 1) * S],
        }
        for core in range(N_CORES)
    ]


def kernel(x: np.ndarray, beta: np.ndarray) -> np.ndarray:
    b, c, h, w = x.shape
    assert (b, c, h, w) == (16, 512, 64, 64), f"unexpected shape {x.shape}"
    hw = h * w
    S = b // N_CORES
    CT = c // P

    nc = _get_program(S, c, hw, N_CORES)
    in_maps = make_in_maps(x)
    res = run_bass_kernel_spmd(nc, in_maps, list(range(N_CORES)))

    y = np.empty((b, P, CT, hw), dtype=NP_BF16)
    for core in range(N_CORES):
        y[core * S : (core + 1) * S] = res.results[core]["out"]
    # [b, P, CT, HW] -> [b, C, HW] fp32
    y = y.transpose(0, 2, 1, 3).astype(np.float32).reshape(b, c, hw)
    # rank-0 epilogue in exact fp32 on the host
    out = np.asarray(x, dtype=np.float32).reshape(b, c, hw) + np.float32(
        np.asarray(beta).reshape(-1)[0]
    ) * y
    return out.reshape(b, c, h, w)


# revision 40
# speedup vs baseline: 1.0220x; 1.0220x over previous
"""Trainium2 Bass kernel for nn_CAM (channel-attention module).

Reference computation per sample (b=16 total):
    xf   = x.reshape(c, h*w)               # [512, 4096]
    attn = softmax(xf @ xf.T, axis=-1)     # [512, 512]
    y    = attn @ xf                       # [512, 4096]
    out  = beta * y + x

Sharding: data-parallel over batch b across 8 NeuronCores (2 samples per
core).  The kernel computes y = softmax(xf xf^T) xf on-chip; the rank-0
epilogue out = x + beta*y runs on the host in fp32 (exact, and it removes
the bf16 x upload + the on-chip elementwise add).

Layout/precision scheme (tolerance 2e-2; matmuls fp8e4 DoubleRow):
  - G = xf xf^T is SYMMETRIC, so P^T[d, c] = exp(G[d,c] - m_c) can be
    built from the natural-layout G tiles with a per-COLUMN max bias --
    no transposes of the 512x512 attention matrix are needed at all.
  - matmul1 (G): xt fp8 (hw-major transpose uploaded from host), 16
    DoubleRow MMs per c-tile; each G tile is row-maxed on the DVE
    (= column max by symmetry) and copied PSUM->SBUF fp32 on the ACT so
    only 2 rotating PSUM banks are needed.  (tensor_tensor_reduce would
    fuse these but crashes the device - verified by bisection.)
  - column-max broadcast: m (shifted by -4096 into bf16 range) is turned
    into M_row[p, c] = m~[c] exactly via 4 diag-mask matmuls
    (lhsT = ones, rhs = identity * m~ per-partition)  -- any bf16
    rounding of m~ cancels between numerator and denominator because the
    row sums are computed FROM the quantized P^T (softmax is invariant
    to a per-row shift applied consistently).
  - P^T tile k = ACT Exp((G_k - 4096) + M_row) -> fp8, values in
    [0, e^~2], safely inside TRN fp8e4 range (max 240).
  - rowsum: 16 N=1 fp8 matmuls P^T(ctile)^T @ ones (~1.3us of PE).  The
    denominator is then the sum of the very fp8 values mm2 multiplies,
    so the P^T quantization cancels in the softmax quotient; an ACT-side
    exp+accum variant was tried and both overloaded the ACT queue (it
    was stalling mm2's drains) and lost the cancellation.
  - matmul2 (y = P @ xf): 2 DR MMs per [128, 512] chunk, 4-chunk groups
    on a stationary weight, two 2-bank PSUM tiles, the first tile's 8 MMs
    emitted before the second's; the drain applies 1/rowsum as a
    per-partition scale on the DVE only (tensor_scalar, bf16 out) -- an
    ACT drain on the PSUM-reuse path repeatedly measured 5-15us slower.
  - PSUM budget: G 2 banks (rotating) + M_row 1 + mm2 4 + rowsum 1 = 8.
  - HBM schedule (per-core DMA BW ~358 GB/s is the scarce resource at
    the start, and each HWDGE queue is packet-rate-bound at ~45
    packets/us): xt is striped across both rings in 4 chunks, x8 rides
    behind it as one 16KB-packet DMA per sample, outputs are one
    8KB-packet DMA per c-tile on alternating rings, and the final
    c-tile ships 1024-col quarters right behind their drains.
  - ~24 identity matmuls run during the initial fill so the PE's HAM
    clock-gate is already at 8/8 when the real matmuls arrive.
  - two-sample software pipeline: sample s+1's matmul1 / sample s's
    matmul2 cover the other sample's softmax tail; emission order is
    arranged so no PE stall exceeds ~0.5us.
"""

import numpy as np
import ml_dtypes

import concourse.bass as bass
import concourse.bacc as bacc
import concourse.mybir as mybir
import concourse.tile as tile
from concourse.bass import ts
from concourse.bass_utils import run_bass_kernel_spmd
from concourse.masks import make_identity

N_CORES = 8
P = 128

F32 = mybir.dt.float32
BF16 = mybir.dt.bfloat16
FP8 = mybir.dt.float8e4

NP_BF16 = ml_dtypes.bfloat16
NP_FP8 = ml_dtypes.float8_e4m3

DR = mybir.MatmulPerfMode.DoubleRow
MULT = mybir.AluOpType.mult
ADD = mybir.AluOpType.add
MAX = mybir.AluOpType.max

# G's diagonal is ||x_c||^2 ~ hw = 4096 for this problem's unit-normal
# input; shifting the column maxes by -4096 keeps them in crisp bf16
# range.  Any residual rounding cancels (see module docstring).
M_SHIFT = 4096.0


def build_program(S=2, C=512, HW=4096, n_cores=N_CORES):
    """Build the SPMD Bass program for one core holding S samples."""
    CT = C // P        # c-tiles (partition tiles of the channel dim)
    NT = HW // P       # n-blocks (contraction tiles for matmul1)
    NCHUNK = 512       # free-dim chunk for matmul2 (one PSUM bank)
    NCH = HW // NCHUNK
    XTC = 4            # xt/x8 arrive in 4 DMA chunks each

    nc = bacc.Bacc(
        "TRN2", target_bir_lowering=False, debug=False, num_devices=n_cores
    )
    # natural x, partition-major, fp8: x8[s, p, i, n] = x[s, 128*i + p, n]
    x8_in = nc.dram_tensor("x8", [S, P, CT, HW], FP8, kind="ExternalInput").ap()
    # transposed x, fp8: xt[s, p, j, c] = x[s, c, 128*j + p]
    xt_in = nc.dram_tensor("xt", [S, P, NT, C], FP8, kind="ExternalInput").ap()
    out_d = nc.dram_tensor("out", [S, P, CT, HW], BF16, kind="ExternalOutput").ap()

    with tile.TileContext(nc) as tc:
        with (
            tc.tile_pool(name="consts", bufs=1) as consts,
            tc.tile_pool(name="xt", bufs=2) as xt_pool,
            tc.tile_pool(name="x8", bufs=2) as x8_pool,
            tc.tile_pool(name="gsb", bufs=2) as gsb_pool,
            tc.tile_pool(name="pt", bufs=2) as pt_pool,
            tc.tile_pool(name="mrow", bufs=2) as mrow_pool,
            tc.tile_pool(name="dsb", bufs=2) as d_pool,
            tc.tile_pool(name="nmd", bufs=8) as nmd_pool,
            tc.tile_pool(name="stats", bufs=2) as stats_pool,
            tc.tile_pool(name="outsb", bufs=3) as out_pool,
            tc.tile_pool(name="psumG", bufs=2, space="PSUM") as psumG_pool,
            # pys[0] is double-buffered (4 banks) so the q0 bank reuse has
            # two groups of drain slack; pys[1] single (2 banks).  M_row,
            # the rowsums and the warmup junk borrow slots from these
            # rotations in the inter-phase windows where mm2 is not live.
            tc.tile_pool(name="psumYA", bufs=2, space="PSUM") as psumYA_pool,
            tc.tile_pool(name="psumYB", bufs=1, space="PSUM") as psumYB_pool,
        ):
            ident = consts.tile([P, P], BF16)
            make_identity(nc, ident[:])
            ones_bf = consts.tile([P, P], BF16)
            nc.vector.memset(ones_bf[:], 1.0)
            ones8 = consts.tile([P, 1], FP8)
            nc.vector.memset(ones8[:], 1.0)

            def warmup_pe(n_mm=24):
                """Identity matmuls during the input fill: trips the HAM
                clock-gate to 8/8 before the real matmul stream starts.
                Output goes to a psumY0-slot tile, long dead before mm2."""
                junk = psumYA_pool.tile(
                    [P, 2, NCHUNK], F32, tag="psumY0", name="junk"
                )
                for _ in range(n_mm):
                    nc.tensor.matmul(
                        junk[:, 0, 0:P], lhsT=ident[:], rhs=ident[:],
                        start=True, stop=True,
                    )

            # per-sample state threaded between phases
            st = [dict() for _ in range(S)]

            # Each DMA queue is packet-rate-bound (~45 pkts/us, one packet
            # per partition-row), so per-queue BW ~= pkt_bytes * 45e6/s.
            # Stripe xt's chunks across both HWDGE rings and keep the
            # packet (= per-partition contiguous run) as big as the
            # consumption order allows; x8 is one 16KB-packet DMA per
            # sample, queued behind xt on its ring so it never steals BW.
            XT_EDGES = [0, 4, 12, 22, NT]

            def load_xt(s, engs=(nc.sync, nc.scalar)):
                xt_t = xt_pool.tile([P, NT, C], FP8, tag="xt")
                for c in range(len(XT_EDGES) - 1):
                    lo, hi = XT_EDGES[c], XT_EDGES[c + 1]
                    engs[c % 2].dma_start(
                        xt_t[:, lo:hi, :], xt_in[s, :, lo:hi, :]
                    )
                st[s]["xt"] = xt_t

            def load_x8(s, eng):
                x8_t = x8_pool.tile([P, CT, HW], FP8, tag="x8", name="x8_t")
                eng.dma_start(x8_t[:, :, :], x8_in[s])
                st[s]["x8"] = x8_t

            def alloc_mm1(s):
                st[s]["gsb"] = gsb_pool.tile([P, CT, C], F32, tag="gsb", name="gsb")
                st[s]["negm"] = stats_pool.tile(
                    [P, CT], F32, tag="negm", name="negm"
                )
                st[s]["nm"] = stats_pool.tile([P, CT], F32, tag="nm", name="nm")

            def mm1_tile(s, i):
                """G c-tile i: 16 DR MMs -> PSUM; rowmax (DVE) + fp32 copy
                (ACT) drain the bank."""
                xt_t, gsb, negm = st[s]["xt"], st[s]["gsb"], st[s]["negm"]
                pa = psumG_pool.tile([P, C], F32, tag="psumG", name="pa")
                for t in range(NT // 2):
                    nc.tensor.matmul(
                        pa[:],
                        lhsT=xt_t[:, 2 * t : 2 * t + 2, ts(i, P)],
                        rhs=xt_t[:, 2 * t : 2 * t + 2, :],
                        start=(t == 0),
                        stop=(t == NT // 2 - 1),
                        perf_mode=DR,
                    )
                # by symmetry rowmax == colmax
                nc.vector.reduce_max(
                    negm[:, i : i + 1], pa[:], axis=mybir.AxisListType.X,
                    negate=True,
                )
                nc.scalar.copy(gsb[:, i, :], pa[:])
                if i == CT - 1:
                    # m~ = bf16-round(M_SHIFT - max) in an fp32 view; any
                    # rounding cancels between P^T and the rowsums because
                    # both derive from the same broadcast values.
                    nm_bf = stats_pool.tile([P, CT], BF16, name="nm_bf")
                    nc.vector.tensor_scalar_add(
                        nm_bf[:], st[s]["negm"][:], M_SHIFT
                    )
                    nc.vector.tensor_copy(st[s]["nm"][:], nm_bf[:])

            def nmd_phase(s):
                """nmd_j = I * nm[:, j] (DVE) - split from the matmuls so
                these can be emitted ahead of mm2's DVE drains."""
                nm = st[s]["nm"]
                nmds = []
                for j in range(CT):
                    nmd = nmd_pool.tile([P, P], BF16, tag="nmd", name="nmd")
                    nc.vector.tensor_scalar_mul(
                        nmd[:], ident[:], nm[:, j : j + 1]
                    )
                    nmds.append(nmd)
                st[s]["nmds"] = nmds

            def mrow_phase(s):
                """M_row[p, c] = nm[c] for all p, via 4 diag-mask matmuls."""
                pm = psumYA_pool.tile(
                    [P, 2, NCHUNK], F32, tag="psumY0", name="pm"
                )
                mrow = mrow_pool.tile([P, C], F32, tag="mrow")
                for j in range(CT):
                    # out[p, q] = sum_k 1 * (I[k,q] * nm[k, j]) = nm[q, j]
                    nc.tensor.matmul(
                        pm[:, 0, ts(j, P)], lhsT=ones_bf[:],
                        rhs=st[s]["nmds"][j][:],
                        start=True, stop=True,
                    )
                nc.vector.tensor_copy(mrow[:], pm[:, 0, :])
                st[s]["mrow"] = mrow

            def softmax_tiles(s, ks=None):
                """PT tile k = exp(G_k - m_col), fp8."""
                gsb, mrow = st[s]["gsb"], st[s]["mrow"]
                if ks is None:
                    ks = range(CT)
                if 0 in ks:
                    st[s]["PT"] = pt_pool.tile(
                        [P, CT, C], FP8, tag="PT", name="PT"
                    )
                PT = st[s]["PT"]
                for k in ks:
                    d_t = d_pool.tile([P, C], BF16, tag="dsb")
                    nc.vector.scalar_tensor_tensor(
                        out=d_t[:],
                        in0=gsb[:, k, :],
                        scalar=-M_SHIFT,
                        in1=mrow[:],
                        op0=ADD,
                        op1=ADD,
                    )
                    nc.scalar.activation(
                        PT[:, k, :], d_t[:], mybir.ActivationFunctionType.Exp
                    )

            def rowsum_phase(s):
                """rowsum[c] = sum_d PT[d, c] via N=1 fp8 matmuls; the
                denominator then exactly matches mm2's quantized numerator."""
                PT = st[s]["PT"]
                rsb = psumYB_pool.tile(
                    [P, 2, NCHUNK], F32, tag="psumY1", name="rsb"
                )
                rs = rsb[:, 0, :]
                rsinv = stats_pool.tile([P, CT], F32, tag="rsinv", name="rsinv")
                for i in range(CT):
                    for k in range(CT):
                        nc.tensor.matmul(
                            rs[:, i : i + 1],
                            lhsT=PT[:, k, ts(i, P)],
                            rhs=ones8[:],
                            start=(k == 0),
                            stop=(k == CT - 1),
                        )
                nc.vector.reciprocal(rsinv[:], rs[:, 0:CT])
                st[s]["rsinv"] = rsinv

            def mm2_phase(s, tiles):
                x8_t, PT = st[s]["x8"], st[s]["PT"]
                last = s == S - 1 and CT - 1 in tiles
                for i in tiles:
                    rsinv = st[s]["rsinv"]
                    fine = last and i == CT - 1   # kernel-tail tile
                    ot = out_pool.tile([P, HW], BF16, tag="outsb")
                    for g in range(NCH // 4):
                        pys = [
                            psumYA_pool.tile(
                                [P, 2, NCHUNK], F32, tag="psumY0", name="py0"
                            ),
                            psumYB_pool.tile(
                                [P, 2, NCHUNK], F32, tag="psumY1", name="py1"
                            ),
                        ]
                        # q-blocked emission: pys[0]'s 8 MMs all run before
                        # pys[1]'s, so each tile's DVE drain starts half a
                        # group early and finishes under the other half's
                        # MMs -- the bank reuse never waits.  Both drains go
                        # to the DVE: the ACT queue's in-order latency was
                        # measured adding ~1.7us/group when it held a drain.
                        for q in range(2):
                            for t in range(CT // 2):
                                for j in range(2):
                                    n = g * 4 + q * 2 + j
                                    nc.tensor.matmul(
                                        pys[q][:, j, :],
                                        lhsT=PT[:, 2 * t : 2 * t + 2, ts(i, P)],
                                        rhs=x8_t[:, 2 * t : 2 * t + 2, ts(n, NCHUNK)],
                                        start=(t == 0),
                                        stop=(t == CT // 2 - 1),
                                        perf_mode=DR,
                                    )
                        # both drains on the DVE: an ACT drain anywhere on
                        # the bank-reuse path measured ~5-15us slower end to
                        # end across three attempts (the ACT's in-order
                        # queue adds ~0.9-1.7us of latency per group).
                        for q in range(2):
                            nc.vector.tensor_scalar_mul(
                                ot[:, ts(2 * g + q, 2 * NCHUNK)],
                                pys[q][:],
                                rsinv[:, i : i + 1],
                            )
                        if fine:
                            # kernel tail: ship each 1024-col quarter as
                            # soon as its drain lands, alternating rings
                            for h in (2 * g, 2 * g + 1):
                                eng = nc.sync if h % 2 == 0 else nc.scalar
                                eng.dma_start(
                                    out_d[s, :, i, ts(h, NCHUNK * 2)],
                                    ot[:, ts(h, NCHUNK * 2)],
                                )
                    if not fine:
                        # one 8KB-packet DMA per c-tile, rings alternating
                        eng = nc.sync if (s * CT + i) % 2 == 0 else nc.scalar
                        eng.dma_start(out_d[s, :, i, :], ot[:])

            # -- software-pipelined emission over the S=2 samples --
            warmup_pe()
            load_xt(0)                     # striped over both rings
            load_xt(1, (nc.scalar, nc.sync))
            load_x8(0, nc.sync)            # FIFO behind xt on each ring
            load_x8(1, nc.scalar)

            alloc_mm1(0)
            for i in range(CT):
                mm1_tile(0, i)
            alloc_mm1(1)
            mm1_tile(1, 0)
            nmd_phase(0)
            mrow_phase(0)          # PE: 4 tiny MMs, hidden under mm1(1)
            softmax_tiles(0)       # DVE+ACT, hidden under mm1(1)
            mm1_tile(1, 1)
            mm1_tile(1, 2)
            mm1_tile(1, 3)
            nmd_phase(1)           # DVE, overlaps the rowsum(0) MMs
            rowsum_phase(0)        # PE: 16 tiny MMs
            mrow_phase(1)          # PE: 4 tiny MMs before mm2 goes hot
            mm2_phase(0, [0, 1])
            # softmax(1)'s DVE subtracts are interleaved between mm2(0)
            # tiles so they never head-of-line-block the mm2 drains
            softmax_tiles(1, ks=[0, 1])
            mm2_phase(0, [2])
            softmax_tiles(1, ks=[2, 3])
            mm2_phase(0, [3])
            rowsum_phase(1)
            mm2_phase(1, [0, 1, 2, 3])

    nc.compile()
    return nc


_PROGRAM_CACHE = {}


def _get_program(S, C, HW, n_cores):
    key = (S, C, HW, n_cores)
    if key not in _PROGRAM_CACHE:
        _PROGRAM_CACHE[key] = build_program(S, C, HW, n_cores)
    return _PROGRAM_CACHE[key]


def make_in_maps(x: np.ndarray):
    """Host-side prep: shard over batch, swizzle + downcast to fp8."""
    b, c, h, w = x.shape
    hw = h * w
    S = b // N_CORES
    CT = c // P
    NT = hw // P

    xf = np.asarray(x, dtype=np.float32).reshape(b, c, hw)
    # natural, partition-major: [b, P, CT, HW]
    x8 = np.ascontiguousarray(
        xf.reshape(b, CT, P, hw).transpose(0, 2, 1, 3)
    ).astype(NP_FP8)
    # transposed: xt[s, p, j, c] = x[s, c, 128j+p] -> [b, P, NT, C]
    xt = np.ascontiguousarray(
        xf.reshape(b, c, NT, P).transpose(0, 3, 2, 1)
    ).astype(NP_FP8)
    return [
        {
            "x8": x8[core * S : (core + 1) * S],
            "xt": xt[core * S : (core + 1) * S],
        }
        for core in range(N_CORES)
    ]


def kernel(x: np.ndarray, beta: np.ndarray) -> np.ndarray:
    b, c, h, w = x.shape
    assert (b, c, h, w) == (16, 512, 64, 64), f"unexpected shape {x.shape}"
    hw = h * w
    S = b // N_CORES
    CT = c // P

    nc = _get_program(S, c, hw, N_CORES)
    in_maps = make_in_maps(x)
    res = run_bass_kernel_spmd(nc, in_maps, list(range(N_CORES)))

    y = np.empty((b, P, CT, hw), dtype=NP_BF16)
    for core in range(N_CORES):
        y[core * S : (core + 1) * S] = res.results[core]["out"]
    # [b, P, CT, HW] -> [b, C, HW] fp32
    y = y.transpose(0, 2, 1, 3).astype(np.float32).reshape(b, c, hw)
    # rank-0 epilogue in exact fp32 on the host
    out = np.asarray(x, dtype=np.float32).reshape(b, c, hw) + np.float32(
        np.asarray(beta).reshape(-1)[0]
    ) * y
    return out.reshape(b, c, h, w)
